# revision 31
# baseline (speedup 1.0000x reference)
"""Multi-head attention (unstabilized softmax) on 8 TRN2 NeuronCores.

Reference computes, per (batch, head):
    scores  = Q @ K^T / sqrt(d)          [S, S]
    weights = exp(scores) / rowsum(exp)  (unstabilized softmax)
    out     = weights @ V                [S, d]

Sharding: B*H = 64 (batch, head) pairs split across 8 cores -> 8 heads per
core, fully independent (no collectives).

Device pipeline per head (S=2048, d=128), q-chunk-major:
  For each q-chunk qc (512 q columns), for each k-tile kt (128 rows):
    mm1 block: scoresT[kt, qc] = K_chunk^T.T @ Q^T[:, qc]  -> PSUM [128, 512]
  Blocks land in 2 ping-ponged PSUM slots of [128, 3, 512] (3 banks each);
  ACT exp consumes 3 blocks per instruction (N=1536; plus one N=512
  remainder per phase) -> bf16 W^T chunks in SBUF.  Larger ACT tiles
  amortize the ~180-cycle per-instruction overhead (ACT is the critical
  engine: exp throughput is 1 elem/lane/cycle @ 1.2 GHz).
  mm2 (interleaved as PE filler): per 128-row q-tile, accumulate over kt:
    po[q,0:129] += W^T[kt,q].T @ [V|1][kt]   (ones col -> rowsum)
  epilogue: DVE reciprocal of col 128, per-partition scalar multiply,
  DMA out f32.  mm2 for q-chunk qc runs during the exp phase of qc+1, so
  the pipeline tail is only the final q-chunk's mm2 (~4us vs ~19us for
  head-major ordering).

Host prep: Q^T/K^T layouts [d, S] in bf16, V augmented with a ones column
([S, 129] bf16) so no device-side casts are needed.
"""

import math
import os

import numpy as np
import ml_dtypes

import concourse.bass as bass  # noqa: F401  (bass types used via APs)
import concourse.mybir as mybir
from concourse import bacc
from concourse.tile import TileContext
from concourse.bass_utils import run_bass_kernel_spmd

B, H, S, D = 4, 16, 2048, 128
N_CORES = 8
HPC = (B * H) // N_CORES  # heads per core
SCALE = 1.0 / math.sqrt(D)

KT = S // 128          # 16 k-tiles per head
QC = S // 512          # 4 q-chunks per head
BLK = 512              # score block = one mm1 matmul (N=512, one PSUM bank)
STITCH = 3             # blocks per exp instruction (3 banks per PSUM slot)

LAST_EXEC_TIME_NS = None
LAST_RESULTS = None
_NC_CACHE = {}


def build(hpc=HPC):
    f32 = mybir.dt.float32
    bf16 = mybir.dt.bfloat16

    nc = bacc.Bacc(None, target_bir_lowering=False)

    # va/out use partition-major DRAM layouts so each partition's DMA run is
    # contiguous (big descriptors; <64KB strided transfers are
    # descriptor-dominated on the SDMA engines).
    qt_d = nc.declare_dram_parameter("qt", [hpc, D, S], bf16, isOutput=False)
    kt_d = nc.declare_dram_parameter("kt", [hpc, D, S], bf16, isOutput=False)
    va_d = nc.declare_dram_parameter("va", [hpc, 128, KT, D + 1], bf16, isOutput=False)
    o_d = nc.declare_dram_parameter("out", [hpc, 128, KT, D], f32, isOutput=True)

    # exp stitch plan per phase: (kt_start, n_blocks).  Each exp(u) window
    # must cover its filler quota plus the NEXT unit's mm1 refill (3 blocks
    # = ~650ns, 2 blocks = ~430ns).  Placing the short 1004ns exps (2-block
    # units) FIRST and LAST means each is followed by a cheap 2-block
    # refill, which raises total per-phase filler capacity above the 68
    # needed (the (3,3,3,3,2,2) order was ~100-140ns short in both 2-block
    # windows, gapping ACT ~240ns per phase).
    plan = [(0, 2), (2, 3), (5, 3), (8, 3), (11, 3), (14, 2)]
    quotas = [5, 13, 13, 13, 16, 8]

    with TileContext(nc) as tc:
        with (
            tc.tile_pool(name="qk", bufs=2) as qk_pool,
            tc.tile_pool(name="va", bufs=2) as va_pool,
            tc.tile_pool(name="wt", bufs=3) as wt_pool,
            tc.tile_pool(name="osb", bufs=4) as osb_pool,
            tc.tile_pool(name="scoreps", bufs=2, space="PSUM") as score_pool,
            tc.tile_pool(name="outps", bufs=2, space="PSUM") as out_ps_pool,
        ):
            head_state = {}

            # HAM pre-warm: the PE clock-gate holds the array at 1.2GHz
            # until ~3.4us of sustained matmul activity.  Burn dummy
            # matmuls on the PE queue while the first input DMAs are in
            # flight (independent sequencers, so this delays nothing) so
            # the first real mm1 runs at the full 2.4GHz.
            warm = qk_pool.tile([128, 128], bf16, tag="warm")
            nc.vector.memset(warm, 0.0)
            wps = score_pool.tile([128, 3, BLK], f32, tag="score")
            for _ in range(40):
                nc.tensor.matmul(
                    out=wps[:, 0, 0:128], lhsT=warm, rhs=warm,
                    start=True, stop=True,
                )

            def load_head(h):
                """DMA head h inputs (bf16, no casts needed)."""
                q_sb = qk_pool.tile([128, S], bf16, tag="q")
                k_sb = qk_pool.tile([128, S], bf16, tag="k")
                if h == 0:
                    # The first exp needs only K cols 0:384 and Q cols
                    # 0:512: issue those two first, on separate DGE rings
                    # (each dma_start is ~600ns of serial sequencer issue).
                    nc.sync.dma_start(out=k_sb[:, 0:512], in_=kt_d[h, :, 0:512])
                    nc.gpsimd.dma_start(out=q_sb[:, 0:512], in_=qt_d[h, :, 0:512])
                    nc.sync.dma_start(out=k_sb[:, 512:S], in_=kt_d[h, :, 512:S])
                    nc.gpsimd.dma_start(out=q_sb[:, 512:S], in_=qt_d[h, :, 512:S])
                else:
                    nc.sync.dma_start(out=k_sb, in_=kt_d[h])
                    nc.gpsimd.dma_start(out=q_sb, in_=qt_d[h])
                va_sb = va_pool.tile([128, KT, D + 1], bf16, tag="va")
                nc.sync.dma_start(out=va_sb, in_=va_d[h])
                return q_sb, k_sb, va_sb

            def mm2_closures(h, qc, wt):
                """Flat list of closures: 64 mm2 matmuls + 4 epilogues.

                Output for the whole phase (4 q-tiles) collects in one SBUF
                tile and ships as a single 256KB store (big descriptors,
                fewer completion waits on the tail).
                """
                _, _, va_sb = head_state[h]
                shared = {}
                out = []
                for qi in range(4):
                    po_box = {}

                    def mk_mm(kt, qi=qi, po_box=po_box):
                        def go():
                            if kt == 0:
                                po = out_ps_pool.tile([128, D + 1], f32, tag="po")
                                po_box["po"] = po
                            nc.tensor.matmul(
                                out=po_box["po"],
                                lhsT=wt[:, kt, qi * 128 : (qi + 1) * 128],
                                rhs=va_sb[:, kt, :],
                                start=(kt == 0),
                                stop=(kt == KT - 1),
                            )
                        return go

                    def mk_epi(qi=qi, po_box=po_box):
                        def go():
                            po = po_box["po"]
                            if qi == 0:
                                o_sb = osb_pool.tile([128, 4, D], f32, tag="osb")
                                shared["o_sb"] = o_sb
                            # In-place reciprocal of the rowsum column, then
                            # one scalar multiply (no separate rc staging).
                            nc.vector.reciprocal(
                                out=po[:, D : D + 1], in_=po[:, D : D + 1]
                            )
                            nc.vector.tensor_scalar_mul(
                                shared["o_sb"][:, qi, :], po[:, 0:D], po[:, D : D + 1]
                            )
                            if h == hpc - 1 and qc == QC - 1:
                                # Final phase: ship each q-tile as its
                                # epilogue completes — the last store's HBM
                                # completion wait gates the teardown drain,
                                # so keep the final transfer small.
                                nc.sync.dma_start(
                                    out=o_d[h, :, qc * 4 + qi, :],
                                    in_=shared["o_sb"][:, qi, :],
                                )
                            elif qi == 3:
                                # Last head's stores on the idle HWDGE queue.
                                store_eng = nc.sync if h == hpc - 1 else nc.gpsimd
                                store_eng.dma_start(
                                    out=o_d[h, :, qc * 4 : (qc + 1) * 4, :],
                                    in_=shared["o_sb"],
                                )
                        return go

                    for kt in range(KT):
                        out.append(mk_mm(kt))
                    out.append(mk_epi())
                return out

            def emit_phase(h, qc, fillers):
                """mm1 + exp for (h, qc); interleave filler closures."""
                if (h, qc) == (0, 0):
                    head_state[0] = load_head(0)
                if qc == QC - 1 and h + 1 < hpc:
                    head_state[h + 1] = load_head(h + 1)
                q_sb, k_sb, _ = head_state[h]
                wt = wt_pool.tile([128, KT, 512], bf16, tag="wt")
                q0 = qc * 512

                fill_iter = iter(fillers)
                for ui, (kt0, nb) in enumerate(plan):
                    ps = score_pool.tile([128, STITCH, BLK], f32, tag="score")
                    for j in range(nb):
                        kt = kt0 + j
                        nc.tensor.matmul(
                            out=ps[:, j, :],
                            lhsT=k_sb[:, kt * 128 : (kt + 1) * 128],
                            rhs=q_sb[:, q0 : q0 + 512],
                            start=True,
                            stop=True,
                        )
                    nc.scalar.activation(
                        out=wt[:, kt0 : kt0 + nb, :],
                        in_=ps[:, 0:nb, :],
                        func=mybir.ActivationFunctionType.Exp,
                        scale=SCALE,
                    )
                    # Interleave mm2 of the previous phase so the PE stream
                    # has matmul work while ACT drains the exp.
                    for _ in range(quotas[ui]):
                        nxt = next(fill_iter, None)
                        if nxt is None:
                            break
                        nxt()
                for nxt in fill_iter:
                    nxt()
                return wt

            prev = None  # (h, qc, wt) awaiting mm2
            for h in range(hpc):
                for qc in range(QC):
                    fillers = mm2_closures(*prev) if prev is not None else []
                    wt = emit_phase(h, qc, fillers)
                    prev = (h, qc, wt)
            for cl in mm2_closures(*prev):
                cl()

    return nc


def _shard_host(Q, K, V, hpc, n_cores):
    """Host-side shard + layout + cast: returns per-core input maps."""
    bf16 = ml_dtypes.bfloat16
    BH = Q.shape[0] * Q.shape[1]
    s, d = Q.shape[2], Q.shape[3]
    kt_n = s // 128
    Qf = Q.reshape(BH, s, d)
    Kf = K.reshape(BH, s, d)
    Vf = V.reshape(BH, s, d)
    # Partition-major [h, p, kt, d+1]: per-partition DMA runs are contiguous.
    Va = np.empty((BH, 128, kt_n, d + 1), dtype=bf16)
    Va[:, :, :, 0:d] = Vf.reshape(BH, kt_n, 128, d).transpose(0, 2, 1, 3).astype(bf16)
    Va[:, :, :, d] = 1.0
    in_maps = []
    for c in range(n_cores):
        sl = slice(c * hpc, (c + 1) * hpc)
        in_maps.append(
            {
                "qt": np.ascontiguousarray(
                    Qf[sl].transpose(0, 2, 1).astype(bf16)
                ),
                "kt": np.ascontiguousarray(
                    Kf[sl].transpose(0, 2, 1).astype(bf16)
                ),
                "va": Va[sl],
            }
        )
    return in_maps


def kernel(Q, K, V):
    global LAST_EXEC_TIME_NS, LAST_RESULTS
    Q = np.asarray(Q, dtype=np.float32)
    K = np.asarray(K, dtype=np.float32)
    V = np.asarray(V, dtype=np.float32)

    trace = os.environ.get("ATTN_TRACE", "0") == "1"

    key = (HPC, S)
    nc = _NC_CACHE.get(key)
    if nc is None:
        nc = build(hpc=HPC)
        nc.compile()
        _NC_CACHE[key] = nc

    in_maps = _shard_host(Q, K, V, HPC, N_CORES)
    res = run_bass_kernel_spmd(nc, in_maps, core_ids=list(range(N_CORES)), trace=trace)
    LAST_EXEC_TIME_NS = res.exec_time_ns
    LAST_RESULTS = res

    # Device out layout is partition-major [hpc, p, qt, d] -> [hpc, S, D].
    out = np.concatenate([res.results[c]["out"] for c in range(N_CORES)], axis=0)
    out = out.reshape(B * H, 128, KT, D).transpose(0, 2, 1, 3)
    return np.ascontiguousarray(out.reshape(B, H, S, D))
